# revision 20
# baseline (speedup 1.0000x reference)
"""Trainium2 Bass kernel for CentroidClassifier (retrieval_knn).

Math (per row x of X[B,D], centers C[Ncls,D]):
    logits  = -0.5*||x-c||^2 = x.c - 0.5*||x||^2 - 0.5*||c||^2
    conf    = softmax(logits)          (rows)
    log_conf= log_softmax(logits)

Strategy: data-parallel over 8 NeuronCores (shard B), replicate centers.
Per core, 64 tiles of 128 rows:
  - PE: transpose the x tile, then compute x @ centersT in PSUM with an
    fp16 hi/lo split (3 cross terms) — ~fp32 accuracy at 1 cyc/col
    instead of fp32's 2 double-cost passes. Two K=1 fp16 matmuls add the
    per-center bias (-0.5*||c||^2) broadcast across rows.
  - softmax is row-wise (free axis): ACT materializes logits (adding the
    per-row -0.5*||x||^2 as a bias, which softmax is invariant to), DVE
    reduce_max must read SBUF (PSUM-source reduces crash this runtime),
    ACT exp with per-row bias and row-sum accumulation.
  - A single ACT table set (natural_log_exp_and_others) covers Identity,
    Copy, Exp and Ln; pin it via BASS_ACT_ROOT_JSON_PATH so walrus does
    not reload tables between Exp and Ln every tile (~2.7us each).
"""

import os

import numpy as np

B, C, D = 65536, 1000, 128
N_CORES = 8
ROWS_PER_CORE = B // N_CORES  # 8192
P = 128
N_TILES = ROWS_PER_CORE // P  # 64
N0 = 512  # PSUM bank split of the C axis: [0,512) | [512,1000)

_CACHE = {}


def _pin_act_tables():
    """Make bass's act-table-set placement resolve every activation to the
    natural_log_exp_and_others set (it contains exp, ln, identity and copy).
    Otherwise Exp and Ln land in different sets and walrus reloads the ACT
    tables (~2.7us) twice per tile. Only the bass-side choice map is
    patched; set ids keep indexing the unmodified act_info.json."""
    import functools

    import concourse.bacc as bacc_mod
    import concourse.hw_specs as hw_specs

    if getattr(hw_specs.get_activation_tables, "_pinned_nle", False):
        return
    orig = hw_specs.get_activation_tables

    @functools.cache
    def pinned(arch):
        full = dict(orig(arch))
        assert "natural_log_exp_and_others" in full
        return {
            name: (funcs if name == "natural_log_exp_and_others" else set())
            for name, funcs in full.items()
        }

    pinned._pinned_nle = True
    hw_specs.get_activation_tables = pinned
    bacc_mod.get_activation_tables = pinned


def _build_program():
    import concourse.bacc as bacc
    import concourse.tile as tile
    from concourse import mybir
    from concourse.masks import make_identity

    _pin_act_tables()

    f32 = mybir.dt.float32
    f16 = mybir.dt.float16
    Alu = mybir.AluOpType
    Act = mybir.ActivationFunctionType
    Ax = mybir.AxisListType

    nc = bacc.Bacc(
        "TRN2", target_bir_lowering=False, debug=False, num_devices=N_CORES
    )

    x_dram = nc.dram_tensor("x", [ROWS_PER_CORE, D], f32, kind="ExternalInput")
    c_dram = nc.dram_tensor("centers", [C, D], f32, kind="ExternalInput")
    logits_dram = nc.dram_tensor(
        "logits", [ROWS_PER_CORE, C], f32, kind="ExternalOutput"
    )
    conf_dram = nc.dram_tensor("conf", [ROWS_PER_CORE, C], f32, kind="ExternalOutput")
    logconf_dram = nc.dram_tensor(
        "log_conf", [ROWS_PER_CORE, C], f32, kind="ExternalOutput"
    )

    CHUNKS = ((0, N0), (N0, C))

    with tile.TileContext(nc) as tc:
        with (
            tc.tile_pool(name="const", bufs=1) as const_pool,
            tc.tile_pool(name="xin", bufs=6) as x_pool,
            tc.tile_pool(name="xt", bufs=6) as xt_pool,
            tc.tile_pool(name="big", bufs=6) as big_pool,
            tc.tile_pool(name="stat", bufs=12) as stat_pool,
            tc.tile_pool(name="psum_g", bufs=3, space="PSUM") as psum_g_pool,
            tc.tile_pool(name="psum_t", bufs=2, space="PSUM") as psum_t_pool,
        ):
            # ---------------- preamble (once per core) ----------------
            identity = const_pool.tile([P, P], f32)
            make_identity(nc, identity[:, :])
            ones_col = const_pool.tile([P, 1], f32)
            nc.vector.memset(ones_col[:, :], 1.0)
            ones2 = const_pool.tile([2, P], f16)
            nc.vector.memset(ones2[:, :], 1.0)

            # centersT[d, c] assembled from PE transposes of [c,d] tiles.
            # One DMA loads all 1000 rows as 8 column-groups of 128.
            n_ct = (C + P - 1) // P  # 8, last group 104 rows
            ct_all = const_pool.tile([P, n_ct, D], f32)
            nc.sync.dma_start(
                out=ct_all[:, : n_ct - 1, :],
                in_=c_dram[: (n_ct - 1) * P, :].rearrange("(j p) d -> p j d", p=P),
            )
            last = C - (n_ct - 1) * P
            nc.sync.dma_start(
                out=ct_all[:last, n_ct - 1, :], in_=c_dram[(n_ct - 1) * P :, :]
            )
            centersT = const_pool.tile([P, C], f32)
            for j in range(n_ct):
                k = j * P
                rows = min(P, C - k)
                pt = psum_t_pool.tile([P, P], f32, tag="tp")
                nc.tensor.transpose(
                    out=pt[:, :rows],
                    in_=ct_all[:rows, j, :],
                    identity=identity[:rows, :rows],
                )
                nc.vector.tensor_copy(out=centersT[:, k : k + rows], in_=pt[:, :rows])

            # fp16 hi/lo split of centersT
            cT_hi = const_pool.tile([P, C], f16)
            nc.vector.tensor_copy(out=cT_hi[:, :], in_=centersT[:, :])
            cT_lo = const_pool.tile([P, C], f16)
            nc.vector.tensor_tensor(
                out=cT_lo[:, :], in0=centersT[:, :], in1=cT_hi[:, :], op=Alu.subtract
            )

            # c_bias[0, c] = -0.5 * sum_d centersT[d, c]^2  (column sums via
            # a ones-vector matmul; DVE cannot reduce across partitions)
            sq_t = const_pool.tile([P, C], f32)
            nc.vector.tensor_tensor(
                out=sq_t[:, :], in0=centersT[:, :], in1=centersT[:, :], op=Alu.mult
            )
            c_bias = const_pool.tile([1, C], f32)
            for j, (a, b) in enumerate(CHUNKS):
                cb_psum = psum_t_pool.tile([1, N0], f32, tag="tp")
                nc.tensor.matmul(
                    cb_psum[0:1, : b - a],
                    ones_col[:, 0:1],
                    sq_t[:, a:b],
                    start=True,
                    stop=True,
                )
                nc.scalar.mul(c_bias[0:1, a:b], cb_psum[0:1, : b - a], -0.5)
            cb_hi = const_pool.tile([1, C], f16)
            nc.vector.tensor_copy(out=cb_hi[:, :], in_=c_bias[:, :])
            cb_lo = const_pool.tile([1, C], f16)
            nc.vector.tensor_tensor(
                out=cb_lo[:, :], in0=c_bias[:, :], in1=cb_hi[:, :], op=Alu.subtract
            )
            # pack [cb_hi; cb_lo] into partitions 0,1 of one tile so a single
            # K=2 ones-matmul applies hi+lo in one pass (DMA moves across
            # partitions; DVE cannot)
            cb_pair = const_pool.tile([2, C], f16)
            nc.sync.dma_start(out=cb_pair[0:1, :], in_=cb_hi[0:1, :])
            nc.sync.dma_start(out=cb_pair[1:2, :], in_=cb_lo[0:1, :])

            # ---------------- main loop: 64 row tiles ----------------
            # software pipeline: loads run 2 tiles ahead, PE transpose + fp16
            # casts 1 tile ahead, so the matmul stream never waits on the
            # transpose->cast->matmul chain.
            x_tiles = {}
            xT_tiles = {}

            def load_x(i):
                r0 = i * P
                x_t = x_pool.tile([P, D], f32)
                nc.gpsimd.dma_start(out=x_t[:, :], in_=x_dram[r0 : r0 + P, :])
                x_tiles[i] = x_t

            def transpose_cast(i):
                x_t = x_tiles[i]
                pt = psum_t_pool.tile([P, P], f32, tag="tp")
                nc.tensor.transpose(
                    out=pt[:, :], in_=x_t[:, :], identity=identity[:, :]
                )
                xT_hi = xt_pool.tile([P, P], f16)
                nc.vector.tensor_copy(out=xT_hi[:, :], in_=pt[:, :])
                xT_lo = xt_pool.tile([P, P], f16)
                nc.vector.tensor_tensor(
                    out=xT_lo[:, :], in0=pt[:, :], in1=xT_hi[:, :], op=Alu.subtract
                )
                xT_tiles[i] = (xT_hi, xT_lo)

            load_x(0)
            load_x(1)
            load_x(2)
            transpose_cast(0)

            for i in range(N_TILES):
                r0 = i * P
                if i + 3 < N_TILES:
                    load_x(i + 3)
                if i + 1 < N_TILES:
                    transpose_cast(i + 1)
                x_t = x_tiles.pop(i)
                xT_hi, xT_lo = xT_tiles.pop(i)

                # nhxsq = -0.5 * row_sum(x^2)
                xsq_scratch = xt_pool.tile([P, D], f32, tag="xsqs")
                nc.vector.tensor_tensor(
                    out=xsq_scratch[:, :], in0=x_t[:, :], in1=x_t[:, :], op=Alu.mult
                )
                xsq = stat_pool.tile([P, 1], f32)
                nc.vector.reduce_sum(
                    out=xsq[:, :], in_=xsq_scratch[:, :], axis=Ax.X
                )
                nhxsq = stat_pool.tile([P, 1], f32)
                nc.vector.tensor_scalar_mul(nhxsq[:, :], xsq[:, :], -0.5)

                # g = x @ centersT - 0.5*||c||^2   (PSUM, 2 banks)
                # fp16 hi/lo: hi.hi + hi.lo + lo.hi (lo.lo ~2^-22, dropped)
                g = psum_g_pool.tile([P, 2, N0], f32)
                g_flat = g.rearrange("p a b -> p (a b)")
                for j, (a, b) in enumerate(CHUNKS):
                    gj = g[:, j, : b - a]
                    nc.tensor.matmul(
                        gj, xT_hi[:, :], cT_hi[:, a:b], start=True, stop=False
                    )
                    nc.tensor.matmul(
                        gj, xT_hi[:, :], cT_lo[:, a:b], start=False, stop=False
                    )
                    nc.tensor.matmul(
                        gj, xT_lo[:, :], cT_hi[:, a:b], start=False, stop=False
                    )
                    nc.tensor.matmul(
                        gj, ones2[0:2, :], cb_pair[0:2, a:b], start=False, stop=True
                    )

                # logits = g - 0.5*||x||^2   (ACT adds the per-row bias while
                # streaming PSUM -> SBUF)
                logits_t = big_pool.tile([P, C], f32)
                nc.scalar.activation(
                    out=logits_t[:, :],
                    in_=g_flat[:, :C],
                    func=Act.Identity,
                    bias=nhxsq[:, :],
                    scale=1.0,
                )

                # negmax = -max(logits) over C (reduce must read SBUF)
                negmax = stat_pool.tile([P, 1], f32)
                nc.vector.reduce_max(
                    out=negmax[:, :], in_=logits_t[:, :], axis=Ax.X, negate=True
                )
                # exp reads g from PSUM: bias = -max(g) = negmax + nhxsq
                bias_exp = stat_pool.tile([P, 1], f32)
                nc.vector.tensor_tensor(
                    out=bias_exp[:, :], in0=negmax[:, :], in1=nhxsq[:, :], op=Alu.add
                )

                # e = exp(g - max_g) = exp(logits - max), s = row_sum(e)
                e_t = big_pool.tile([P, C], f32)
                s_sum = stat_pool.tile([P, 1], f32)
                nc.scalar.activation(
                    out=e_t[:, :],
                    in_=g_flat[:, :C],
                    func=Act.Exp,
                    bias=bias_exp[:, :],
                    scale=1.0,
                    accum_out=s_sum[:, :],
                )

                ln_s = stat_pool.tile([P, 1], f32)
                nc.scalar.activation(out=ln_s[:, :], in_=s_sum[:, :], func=Act.Ln)
                recip = stat_pool.tile([P, 1], f32)
                nc.vector.reciprocal(out=recip[:, :], in_=s_sum[:, :])
                # m2 = negmax - ln_s ; log_conf = logits + m2
                m2 = stat_pool.tile([P, 1], f32)
                nc.vector.tensor_scalar(
                    m2[:, :],
                    negmax[:, :],
                    ln_s[:, :],
                    None,
                    Alu.subtract,
                )

                lc_t = big_pool.tile([P, C], f32)
                nc.vector.tensor_scalar_add(lc_t[:, :], logits_t[:, :], m2[:, :])
                # conf = e / s  (in place)
                nc.vector.tensor_scalar_mul(e_t[:, :], e_t[:, :], recip[:, :])

                nc.sync.dma_start(out=logits_dram[r0 : r0 + P, :], in_=logits_t[:, :])
                nc.gpsimd.dma_start(out=conf_dram[r0 : r0 + P, :], in_=e_t[:, :])
                nc.sync.dma_start(out=logconf_dram[r0 : r0 + P, :], in_=lc_t[:, :])

    nc.compile()
    return nc


def _get_program():
    if "nc" not in _CACHE:
        _CACHE["nc"] = _build_program()
    return _CACHE["nc"]


def kernel(x, centers, _trace=False):
    from concourse.bass_utils import run_bass_kernel_spmd

    x = np.ascontiguousarray(np.asarray(x, dtype=np.float32))
    centers = np.ascontiguousarray(np.asarray(centers, dtype=np.float32))
    assert x.shape == (B, D) and centers.shape == (C, D)

    nc = _get_program()
    in_maps = [
        {
            "x": x[k * ROWS_PER_CORE : (k + 1) * ROWS_PER_CORE],
            "centers": centers,
        }
        for k in range(N_CORES)
    ]
    res = run_bass_kernel_spmd(
        nc, in_maps, core_ids=list(range(N_CORES)), trace=_trace
    )
    _CACHE["last_res"] = res
    logits = np.concatenate([r["logits"] for r in res.results], axis=0)
    conf = np.concatenate([r["conf"] for r in res.results], axis=0)
    log_conf = np.concatenate([r["log_conf"] for r in res.results], axis=0)
    return logits, conf, log_conf
